# revision 8
# baseline (speedup 1.0000x reference)
"""Trainium2 Bass kernel for nn_CrossLayerLight (gnn_message_passing).

Strategy (8 NeuronCores):
  Launch A: 8 cores = 2 batches x 2 directions x 2 query-halves.
    Each core: 4096 queries vs 8192 candidates.
      - PE: -d^2 via augmented K=5 matmul (2x1.x2 - |x1|^2 - |x2|^2)
      - DVE: exact top-16 via per-1024-chunk top-8 (max8+max_index) +
        per-row threshold + masked-index extraction (max8 rounds)
      - GPSIMD ap_gather: channel-major neighbor feature gather
      - PE MLP with block-diagonal weights (two query-halves packed into
        128 partitions); leaky = 0.9*relu + 0.1*identity folded into
        doubled matmuls; max-pool over K directly from PSUM.
    Produces fn1, fn2 (projected) and the (x1->x2) knn indices.
  Launch B: 8 cores = 2 batches x 4 query-shards of cross-layer 3,
    reusing launch-A indices and fn1/fn2.
"""

import sys

sys.path.insert(0, "/opt/trn_rl_repo")

import numpy as np

import concourse.bacc as bacc
import concourse.bass as bass
import concourse.mybir as mybir
from concourse.tile import TileContext

F32 = mybir.dt.float32
I16 = mybir.dt.int16
U32 = mybir.dt.uint32

LEAKY = 0.1
K = 16
C = 64
QB = 128          # queries per block (partition dim)
CHUNK = 1024      # candidate chunk for L1 top-8
IDX_OFF = 16384.0  # offset so masked-out candidates (0) never win


def _bd(w):
    """[64,64] -> [128,128] block-diagonal (two copies)."""
    out = np.zeros((128, 128), np.float32)
    out[:64, :64] = w
    out[64:, 64:] = w
    return out


def _tile_bias(b):
    return np.tile(np.asarray(b, np.float32).reshape(-1), 2).reshape(128, 1)


def build_launch_a(nq=4096, ncand=8192):
    """Bass program: distance + topk + cross MLP (2 layers) + projection."""
    nblk = nq // QB
    nchunk = ncand // CHUNK
    ncand8 = 8 * nchunk
    sw = min(512, nq)
    pgrp = min(8, nblk)

    nc = bacc.Bacc("TRN2", target_bir_lowering=False, num_devices=8)

    # inputs
    q_aug = nc.dram_tensor("q_aug", [5, nq], F32, kind="ExternalInput").ap()
    c_aug = nc.dram_tensor("c_aug", [5, ncand], F32, kind="ExternalInput").ap()
    q_feat = nc.dram_tensor("q_feat", [C, nq], F32, kind="ExternalInput").ap()
    c_feat = nc.dram_tensor("c_feat", [C, ncand], F32, kind="ExternalInput").ap()
    t11_wT = nc.dram_tensor("t11_wT", [C, C], F32, kind="ExternalInput").ap()
    t22_wT = nc.dram_tensor("t22_wT", [C, C], F32, kind="ExternalInput").ap()
    pos_wT = nc.dram_tensor("pos_wT", [3, C], F32, kind="ExternalInput").ap()
    npos_h = nc.dram_tensor("npos_h", [3, C], F32, kind="ExternalInput").ap()
    bias_comb = nc.dram_tensor("bias_comb", [128, 1], F32, kind="ExternalInput").ap()
    w1s9 = nc.dram_tensor("w1s9", [128, 128], F32, kind="ExternalInput").ap()
    w1s1 = nc.dram_tensor("w1s1", [128, 128], F32, kind="ExternalInput").ap()
    w2s9 = nc.dram_tensor("w2s9", [128, 128], F32, kind="ExternalInput").ap()
    w2s1 = nc.dram_tensor("w2s1", [128, 128], F32, kind="ExternalInput").ap()
    b1_t = nc.dram_tensor("b1_t", [128, 1], F32, kind="ExternalInput").ap()
    b2_t = nc.dram_tensor("b2_t", [128, 1], F32, kind="ExternalInput").ap()
    projT_bd = nc.dram_tensor("projT_bd", [128, 128], F32, kind="ExternalInput").ap()
    projb_t = nc.dram_tensor("projb_t", [128, 1], F32, kind="ExternalInput").ap()
    i128 = nc.dram_tensor("i128", [128, 128], F32, kind="ExternalInput").ap()
    rep16 = nc.dram_tensor("rep16", [16, C], F32, kind="ExternalInput").ap()
    offs = nc.dram_tensor("offs", [128, ncand8], F32, kind="ExternalInput").ap()

    fn_out = nc.dram_tensor("fn_out", [C, nq], F32, kind="ExternalOutput").ap()
    idx_out = nc.dram_tensor("idx_out", [nq, K], F32, kind="ExternalOutput").ap()

    with TileContext(nc) as tc:
        with (
            tc.tile_pool(name="big", bufs=1) as big,
            tc.tile_pool(name="wpool", bufs=1) as wp,
            tc.tile_pool(name="work", bufs=2) as work,
        ):
            # ---- persistent SBUF ----
            q_aug_sb = big.tile([5, nq], F32, tag="qaug")
            c_aug_sb = big.tile([5, ncand], F32, tag="caug")
            s2_sb = big.tile([128, ncand], F32, tag="s2")
            t1n_sb = big.tile([128, nq], F32, tag="t1n")
            idxw_all = big.tile([128, nblk * C], I16, tag="idxw")

            nc.sync.dma_start(out=q_aug_sb[:], in_=q_aug)
            nc.sync.dma_start(out=c_aug_sb[:], in_=c_aug)

            i128_sb = wp.tile([128, 128], F32, tag="i128")
            rep16_sb = wp.tile([16, C], F32, tag="rep16")
            offs_sb = wp.tile([128, ncand8], F32, tag="offs")
            w1s9_sb = wp.tile([128, 128], F32, tag="w1s9")
            w1s1_sb = wp.tile([128, 128], F32, tag="w1s1")
            w2s9_sb = wp.tile([128, 128], F32, tag="w2s9")
            w2s1_sb = wp.tile([128, 128], F32, tag="w2s1")
            projT_sb = wp.tile([128, 128], F32, tag="projT")
            b1_sb = wp.tile([128, 1], F32, tag="b1")
            b2_sb = wp.tile([128, 1], F32, tag="b2")
            projb_sb = wp.tile([128, 1], F32, tag="projb")
            bcomb_sb = wp.tile([128, 1], F32, tag="bcomb")
            t11_sb = wp.tile([C, C], F32, tag="t11")
            t22_sb = wp.tile([C, C], F32, tag="t22")
            posw_sb = wp.tile([3, C], F32, tag="posw")
            nposh_sb = wp.tile([3, C], F32, tag="nposh")
            for dst, src in [
                (i128_sb, i128), (rep16_sb, rep16), (offs_sb, offs),
                (w1s9_sb, w1s9), (w1s1_sb, w1s1), (w2s9_sb, w2s9),
                (w2s1_sb, w2s1), (projT_sb, projT_bd), (b1_sb, b1_t),
                (b2_sb, b2_t), (projb_sb, projb_t), (bcomb_sb, bias_comb),
                (t11_sb, t11_wT), (t22_sb, t22_wT), (posw_sb, pos_wT),
                (nposh_sb, npos_h),
            ]:
                nc.sync.dma_start(out=dst[:], in_=src)

            # ---- setup: s2 table (cand feat + cand pos proj), both halves ----
            with tc.tile_pool(name="setup", bufs=1) as setup, \
                 tc.tile_pool(name="setps", bufs=2, space="PSUM") as setps:
                c_feat_sb = setup.tile([C, ncand], F32, tag="cfeat")
                q_feat_sb = setup.tile([C, nq], F32, tag="qfeat")
                nc.sync.dma_start(out=c_feat_sb[:], in_=c_feat)
                nc.sync.dma_start(out=q_feat_sb[:], in_=q_feat)
                for j in range(ncand // 512):
                    sl = slice(512 * j, 512 * j + 512)
                    ps = setps.tile([C, 512], F32, tag="sps")
                    nc.tensor.matmul(out=ps[:], lhsT=t22_sb[:],
                                     rhs=c_feat_sb[:, sl], start=True, stop=False)
                    nc.tensor.matmul(out=ps[:], lhsT=posw_sb[:],
                                     rhs=c_aug_sb[0:3, sl], start=False, stop=True)
                    nc.vector.tensor_copy(out=s2_sb[0:64, sl], in_=ps[:])
                # t1n = t11(q_feat) - 0.5*pos( 2x1 ) + bias_comb
                for j in range(nq // sw):
                    sl = slice(sw * j, sw * j + sw)
                    ps = setps.tile([C, sw], F32, tag="sps")
                    nc.tensor.matmul(out=ps[:], lhsT=t11_sb[:],
                                     rhs=q_feat_sb[:, sl], start=True, stop=False)
                    nc.tensor.matmul(out=ps[:], lhsT=nposh_sb[:],
                                     rhs=q_aug_sb[0:3, sl], start=False, stop=True)
                    nc.vector.tensor_scalar(out=t1n_sb[0:64, sl], in0=ps[:],
                                            scalar1=bcomb_sb[0:64, :], scalar2=None,
                                            op0=mybir.AluOpType.add)
            # replicate to partitions 64:128 (t1n shifted by 64 queries)
            nc.sync.dma_start(out=s2_sb[64:128, :], in_=s2_sb[0:64, :])
            nc.sync.dma_start(out=t1n_sb[64:128, 0:nq - 64], in_=t1n_sb[0:64, 64:nq])

            mainps = tc.tile_pool(name="distp", bufs=2, space="PSUM")
            distp = mainps.__enter__()
            mlpp_cm = tc.tile_pool(name="mlpp", bufs=2, space="PSUM")
            mlpp = mlpp_cm.__enter__()
            smallp_cm = tc.tile_pool(name="smallp", bufs=2, space="PSUM")
            smallp = smallp_cm.__enter__()
            pooled_col = work.tile([128, C * pgrp], F32, tag="pooled")
            # ---- main loop over query blocks ----
            for bi in range(nblk):
                q0 = QB * bi
                lhs_q = q_aug_sb[:, q0:q0 + QB]
                # --- distances + L1 top8 per chunk ---
                vals = work.tile([128, ncand8], F32, tag="vals")
                pidx = work.tile([128, ncand8], U32, tag="pidx")
                for cki in range(nchunk):
                    dps = distp.tile([128, CHUNK], F32, tag="dps")
                    for h in range(CHUNK // 512):
                        c0 = CHUNK * cki + 512 * h
                        nc.tensor.matmul(out=dps[:, 512 * h:512 * h + 512],
                                         lhsT=lhs_q,
                                         rhs=c_aug_sb[:, c0:c0 + 512],
                                         start=True, stop=True)
                    nc.vector.max(out=vals[:, 8 * cki:8 * cki + 8], in_=dps[:])
                    nc.vector.max_index(out=pidx[:, 8 * cki:8 * cki + 8],
                                        in_max=vals[:, 8 * cki:8 * cki + 8],
                                        in_values=dps[:])
                # --- L2: threshold = 16th largest of the L1 candidates ---
                gidx = work.tile([128, ncand8], F32, tag="gidx")
                nc.vector.tensor_copy(out=gidx[:], in_=pidx[:])
                nc.vector.tensor_tensor(out=gidx[:], in0=gidx[:], in1=offs_sb[:],
                                        op=mybir.AluOpType.add)
                r8 = work.tile([128, 16], F32, tag="r8")
                vrep = work.tile([128, ncand8], F32, tag="vrep")
                nc.vector.max(out=r8[:, 0:8], in_=vals[:])
                nc.vector.match_replace(out=vrep[:], in_to_replace=r8[:, 0:8],
                                        in_values=vals[:], imm_value=-3.0e38)
                nc.vector.max(out=r8[:, 8:16], in_=vrep[:])
                # mask of >= T ; masked global indices (invalid -> 0)
                msk = work.tile([128, ncand8], F32, tag="msk")
                nc.vector.tensor_scalar(out=msk[:], in0=vals[:],
                                        scalar1=r8[:, 15:16], scalar2=None,
                                        op0=mybir.AluOpType.is_ge)
                mgi = work.tile([128, ncand8], F32, tag="mgi")
                nc.vector.tensor_tensor(out=mgi[:], in0=msk[:], in1=gidx[:],
                                        op=mybir.AluOpType.mult)
                e16 = work.tile([128, 16], F32, tag="e16")
                nc.vector.max(out=e16[:, 0:8], in_=mgi[:])
                nc.vector.match_replace(out=mgi[:], in_to_replace=e16[:, 0:8],
                                        in_values=mgi[:], imm_value=0.0)
                nc.vector.max(out=e16[:, 8:16], in_=mgi[:])
                idx16 = work.tile([128, 16], F32, tag="idx16")
                nc.vector.tensor_scalar(out=idx16[:], in0=e16[:],
                                        scalar1=-IDX_OFF, scalar2=None,
                                        op0=mybir.AluOpType.add)
                nc.sync.dma_start(out=idx_out[q0:q0 + QB, :], in_=idx16[:])
                # --- wrap indices into gpsimd layout: [16k, q] replicated 4x ---
                trp = smallp.tile([16, 128], F32, tag="sm")
                nc.tensor.transpose(out=trp[:], in_=idx16[:], identity=i128_sb[:])
                trs = work.tile([16, 128], F32, tag="trs")
                nc.vector.tensor_copy(out=trs[:], in_=trp[:])
                iwp = smallp.tile([128, C], F32, tag="sm")
                nc.tensor.matmul(out=iwp[0:64, :], lhsT=rep16_sb[:],
                                 rhs=trs[:, 0:64], start=True, stop=True)
                nc.tensor.matmul(out=iwp[64:128, :], lhsT=rep16_sb[:],
                                 rhs=trs[:, 64:128], start=True, stop=True,
                                 tile_position=(0, 64))
                nc.vector.tensor_copy(out=idxw_all[:, C * bi:C * bi + C], in_=iwp[:])
                # --- gather neighbor features (channel-major, both halves) ---
                gat = work.tile([128, CHUNK], F32, tag="gat")
                nc.gpsimd.ap_gather(out_ap=gat[:], in_ap=s2_sb[:],
                                    idxs_ap=idxw_all[:, C * bi:C * bi + C],
                                    channels=128, num_elems=ncand, d=1,
                                    num_idxs=CHUNK)
                # --- MLP ---
                t1n_rep = (t1n_sb[:, q0:q0 + 64]
                           .unsqueeze(2).broadcast_to([128, 64, 16]))
                for h in range(2):
                    fs = slice(512 * h, 512 * h + 512)
                    u1 = mlpp.tile([128, 512], F32, tag="u")
                    nc.tensor.matmul(out=u1[:], lhsT=i128_sb[:], rhs=gat[:, fs],
                                     start=True, stop=False)
                    rr = t1n_rep[:, 32 * h:32 * h + 32, :]
                    nc.tensor.matmul(out=u1[:], lhsT=i128_sb[:], rhs=rr,
                                     start=False, stop=True)
                    r1 = work.tile([128, 512], F32, tag="r1")
                    c1 = work.tile([128, 512], F32, tag="c1")
                    nc.scalar.activation(r1[:], u1[:],
                                         mybir.ActivationFunctionType.Relu)
                    nc.scalar.activation(c1[:], u1[:],
                                         mybir.ActivationFunctionType.Copy)
                    u2 = mlpp.tile([128, 512], F32, tag="u")
                    nc.tensor.matmul(out=u2[:], lhsT=w1s9_sb[:], rhs=r1[:],
                                     start=True, stop=False)
                    nc.tensor.matmul(out=u2[:], lhsT=w1s1_sb[:], rhs=c1[:],
                                     start=False, stop=True)
                    r2 = work.tile([128, 512], F32, tag="r2")
                    c2 = work.tile([128, 512], F32, tag="c2")
                    nc.scalar.activation(r2[:], u2[:],
                                         mybir.ActivationFunctionType.Relu,
                                         bias=b1_sb[:, :])
                    nc.scalar.activation(c2[:], u2[:],
                                         mybir.ActivationFunctionType.Identity,
                                         bias=b1_sb[:, :])
                    u3 = mlpp.tile([128, 512], F32, tag="u")
                    nc.tensor.matmul(out=u3[:], lhsT=w2s9_sb[:], rhs=r2[:],
                                     start=True, stop=False)
                    nc.tensor.matmul(out=u3[:], lhsT=w2s1_sb[:], rhs=c2[:],
                                     start=False, stop=True)
                    # max-pool over K directly from PSUM, then leaky(x + b2)
                    pp = work.tile([128, 32], F32, tag="pp")
                    nc.vector.tensor_reduce(out=pp[:],
                                            in_=u3[:].rearrange("p (q k) -> p q k",
                                                                k=16),
                                            axis=mybir.AxisListType.X,
                                            op=mybir.AluOpType.max)
                    pb = work.tile([128, 32], F32, tag="pb")
                    nc.vector.tensor_scalar(out=pb[:], in0=pp[:],
                                            scalar1=b2_sb[:, :], scalar2=None,
                                            op0=mybir.AluOpType.add)
                    po = pooled_col[:, C * (bi % pgrp) + 32 * h:
                                    C * (bi % pgrp) + 32 * h + 32]
                    nc.vector.tensor_scalar(out=po, in0=pb[:],
                                            scalar1=LEAKY, scalar2=None,
                                            op0=mybir.AluOpType.mult)
                    nc.vector.tensor_tensor(out=po, in0=po, in1=pb[:],
                                            op=mybir.AluOpType.max)
                # --- projection every pgrp blocks ---
                if bi % pgrp == pgrp - 1:
                    g8 = bi // pgrp
                    prj = smallp.tile([128, C * pgrp], F32, tag="sm")
                    nc.tensor.matmul(out=prj[:], lhsT=projT_sb[:],
                                     rhs=pooled_col[:], start=True, stop=True)
                    fno = work.tile([128, C * pgrp], F32, tag="fno")
                    nc.vector.tensor_scalar(out=fno[:], in0=prj[:],
                                            scalar1=projb_sb[:, :], scalar2=None,
                                            op0=mybir.AluOpType.add)
                    # halves interleave: block b covers queries [128b,128b+128):
                    # partitions 0:64 -> q 128b..+64 ; 64:128 -> q 128b+64..+128
                    dstA = (fn_out[:, QB * pgrp * g8:QB * pgrp * (g8 + 1)]
                            .rearrange("c (b q) -> c b q", q=QB)[:, :, 0:64])
                    dstB = (fn_out[:, QB * pgrp * g8:QB * pgrp * (g8 + 1)]
                            .rearrange("c (b q) -> c b q", q=QB)[:, :, 64:128])
                    srcA = fno[0:64, :].rearrange("c (b q) -> c b q", q=64)
                    srcB = fno[64:128, :].rearrange("c (b q) -> c b q", q=64)
                    nc.sync.dma_start(out=dstA, in_=srcA)
                    nc.sync.dma_start(out=dstB, in_=srcB)
                    pooled_col = work.tile([128, C * pgrp], F32, tag="pooled")
            smallp_cm.__exit__(None, None, None)
            mlpp_cm.__exit__(None, None, None)
            mainps.__exit__(None, None, None)
    nc.compile()
    return nc


def build_launch_b(nq=2048, ncand=8192):
    """Bass program: cross-layer 3 (gather + 1 mlp layer + pool)."""
    nblk = nq // QB
    sw = min(512, nq)
    pgrp = min(8, nblk)
    nc = bacc.Bacc("TRN2", target_bir_lowering=False, num_devices=8)

    tab_feat = nc.dram_tensor("tab_feat", [C, ncand], F32, kind="ExternalInput").ap()
    c_pc = nc.dram_tensor("c_pc", [3, ncand], F32, kind="ExternalInput").ap()
    q_fn = nc.dram_tensor("q_fn", [C, nq], F32, kind="ExternalInput").ap()
    q_pc = nc.dram_tensor("q_pc", [3, nq], F32, kind="ExternalInput").ap()
    idxw = nc.dram_tensor("idxw", [128, nq // QB * C], I16,
                          kind="ExternalInput").ap()
    pos_wT = nc.dram_tensor("pos_wT", [3, C], F32, kind="ExternalInput").ap()
    npos_wT = nc.dram_tensor("npos_wT", [3, C], F32, kind="ExternalInput").ap()
    posb_t = nc.dram_tensor("posb_t", [128, 1], F32, kind="ExternalInput").ap()
    w1s9 = nc.dram_tensor("w1s9", [128, 128], F32, kind="ExternalInput").ap()
    w1s1 = nc.dram_tensor("w1s1", [128, 128], F32, kind="ExternalInput").ap()
    b1_t = nc.dram_tensor("b1_t", [128, 1], F32, kind="ExternalInput").ap()
    i128 = nc.dram_tensor("i128", [128, 128], F32, kind="ExternalInput").ap()
    i64 = nc.dram_tensor("i64", [C, C], F32, kind="ExternalInput").ap()

    ff_out = nc.dram_tensor("ff_out", [C, nq], F32, kind="ExternalOutput").ap()

    with TileContext(nc) as tc:
        with (
            tc.tile_pool(name="big", bufs=1) as big,
            tc.tile_pool(name="wpool", bufs=1) as wp,
            tc.tile_pool(name="work", bufs=2) as work,
            tc.tile_pool(name="mlpp", bufs=2, space="PSUM") as mlpp,
        ):
            tab_sb = big.tile([128, ncand], F32, tag="tab")
            t1n_sb = big.tile([128, nq], F32, tag="t1n")
            idxw_sb = big.tile([128, nblk * C], I16, tag="idxw")
            ffc = big.tile([128, C * pgrp], F32, tag="ffc")
            nc.sync.dma_start(out=idxw_sb[:], in_=idxw)

            i128_sb = wp.tile([128, 128], F32, tag="i128")
            i64_sb = wp.tile([C, C], F32, tag="i64")
            w1s9_sb = wp.tile([128, 128], F32, tag="w1s9")
            w1s1_sb = wp.tile([128, 128], F32, tag="w1s1")
            b1_sb = wp.tile([128, 1], F32, tag="b1")
            posb_sb = wp.tile([128, 1], F32, tag="posb")
            posw_sb = wp.tile([3, C], F32, tag="posw")
            nposw_sb = wp.tile([3, C], F32, tag="nposw")
            for dst, src in [(i128_sb, i128), (i64_sb, i64), (w1s9_sb, w1s9),
                             (w1s1_sb, w1s1), (b1_sb, b1_t), (posb_sb, posb_t),
                             (posw_sb, pos_wT), (nposw_sb, npos_wT)]:
                nc.sync.dma_start(out=dst[:], in_=src)

            with tc.tile_pool(name="setup", bufs=1) as setup, \
                 tc.tile_pool(name="setps", bufs=2, space="PSUM") as setps:
                tf_sb = setup.tile([C, ncand], F32, tag="tf")
                cpc_sb = setup.tile([3, ncand], F32, tag="cpc")
                qfn_sb = setup.tile([C, nq], F32, tag="qfn")
                qpc_sb = setup.tile([3, nq], F32, tag="qpc")
                nc.sync.dma_start(out=tf_sb[:], in_=tab_feat)
                nc.sync.dma_start(out=cpc_sb[:], in_=c_pc)
                nc.sync.dma_start(out=qfn_sb[:], in_=q_fn)
                nc.sync.dma_start(out=qpc_sb[:], in_=q_pc)
                for j in range(ncand // 512):
                    sl = slice(512 * j, 512 * j + 512)
                    ps = setps.tile([C, 512], F32, tag="sps")
                    nc.tensor.matmul(out=ps[:], lhsT=i64_sb[:], rhs=tf_sb[:, sl],
                                     start=True, stop=False)
                    nc.tensor.matmul(out=ps[:], lhsT=posw_sb[:], rhs=cpc_sb[:, sl],
                                     start=False, stop=True)
                    nc.vector.tensor_copy(out=tab_sb[0:64, sl], in_=ps[:])
                for j in range(nq // sw):
                    sl = slice(sw * j, sw * j + sw)
                    ps = setps.tile([C, sw], F32, tag="sps")
                    nc.tensor.matmul(out=ps[:], lhsT=i64_sb[:], rhs=qfn_sb[:, sl],
                                     start=True, stop=False)
                    nc.tensor.matmul(out=ps[:], lhsT=nposw_sb[:], rhs=qpc_sb[:, sl],
                                     start=False, stop=True)
                    nc.vector.tensor_scalar(out=t1n_sb[0:64, sl], in0=ps[:],
                                            scalar1=posb_sb[0:64, :], scalar2=None,
                                            op0=mybir.AluOpType.add)
            nc.sync.dma_start(out=tab_sb[64:128, :], in_=tab_sb[0:64, :])
            nc.sync.dma_start(out=t1n_sb[64:128, 0:nq - 64], in_=t1n_sb[0:64, 64:nq])

            for bi in range(nblk):
                q0 = QB * bi
                gat = work.tile([128, CHUNK], F32, tag="gat")
                nc.gpsimd.ap_gather(out_ap=gat[:], in_ap=tab_sb[:],
                                    idxs_ap=idxw_sb[:, C * bi:C * bi + C],
                                    channels=128, num_elems=ncand, d=1,
                                    num_idxs=CHUNK)
                t1n_rep = (t1n_sb[:, q0:q0 + 64]
                           .unsqueeze(2).broadcast_to([128, 64, 16]))
                for h in range(2):
                    fs = slice(512 * h, 512 * h + 512)
                    u1 = mlpp.tile([128, 512], F32, tag="u")
                    nc.tensor.matmul(out=u1[:], lhsT=i128_sb[:], rhs=gat[:, fs],
                                     start=True, stop=False)
                    rr = t1n_rep[:, 32 * h:32 * h + 32, :]
                    nc.tensor.matmul(out=u1[:], lhsT=i128_sb[:], rhs=rr,
                                     start=False, stop=True)
                    r1 = work.tile([128, 512], F32, tag="r1")
                    c1 = work.tile([128, 512], F32, tag="c1")
                    nc.scalar.activation(r1[:], u1[:],
                                         mybir.ActivationFunctionType.Relu)
                    nc.scalar.activation(c1[:], u1[:],
                                         mybir.ActivationFunctionType.Copy)
                    u2 = mlpp.tile([128, 512], F32, tag="u")
                    nc.tensor.matmul(out=u2[:], lhsT=w1s9_sb[:], rhs=r1[:],
                                     start=True, stop=False)
                    nc.tensor.matmul(out=u2[:], lhsT=w1s1_sb[:], rhs=c1[:],
                                     start=False, stop=True)
                    pp = work.tile([128, 32], F32, tag="pp")
                    nc.vector.tensor_reduce(out=pp[:],
                                            in_=u2[:].rearrange("p (q k) -> p q k",
                                                                k=16),
                                            axis=mybir.AxisListType.X,
                                            op=mybir.AluOpType.max)
                    pb = work.tile([128, 32], F32, tag="pb")
                    nc.vector.tensor_scalar(out=pb[:], in0=pp[:],
                                            scalar1=b1_sb[:, :], scalar2=None,
                                            op0=mybir.AluOpType.add)
                    po = ffc[:, C * (bi % pgrp) + 32 * h:
                             C * (bi % pgrp) + 32 * h + 32]
                    nc.vector.tensor_scalar(out=po, in0=pb[:], scalar1=LEAKY,
                                            scalar2=None, op0=mybir.AluOpType.mult)
                    nc.vector.tensor_tensor(out=po, in0=po, in1=pb[:],
                                            op=mybir.AluOpType.max)
                if bi % pgrp == pgrp - 1:
                    g8 = bi // pgrp
                    dstA = (ff_out[:, QB * pgrp * g8:QB * pgrp * (g8 + 1)]
                            .rearrange("c (b q) -> c b q", q=QB)[:, :, 0:64])
                    dstB = (ff_out[:, QB * pgrp * g8:QB * pgrp * (g8 + 1)]
                            .rearrange("c (b q) -> c b q", q=QB)[:, :, 64:128])
                    srcA = ffc[0:64, :].rearrange("c (b q) -> c b q", q=64)
                    srcB = ffc[64:128, :].rearrange("c (b q) -> c b q", q=64)
                    nc.sync.dma_start(out=dstA, in_=srcA)
                    nc.sync.dma_start(out=dstB, in_=srcB)
                    ffc = big.tile([128, C * pgrp], F32, tag="ffc")
    nc.compile()
    return nc


# ------------------------- host orchestration -------------------------

def _aug_q(pc):
    # [3, n] -> [5, n]: rows 2x,2y,2z, -|x|^2, 1
    n2 = (pc.astype(np.float32) ** 2).sum(axis=0)
    return np.concatenate([2.0 * pc, -n2[None, :],
                           np.ones_like(n2)[None, :]], axis=0).astype(np.float32)


def _aug_c(pc):
    # [3, n] -> [5, n]: rows x,y,z, 1, -|x|^2
    n2 = (pc.astype(np.float32) ** 2).sum(axis=0)
    return np.concatenate([pc, np.ones_like(n2)[None, :],
                           -n2[None, :]], axis=0).astype(np.float32)


def _wrap_idx(idx):
    """[nq, 16] int -> gpsimd wrapped layout [128, nq*16//2048*64] int16.

    Block b (128 queries): col block [64b, 64b+64); partition 16g+p,
    col s of block: g in 0..3 -> idx[128b + s, p] ; g in 4..7 ->
    idx[128b + 64 + s, p].
    """
    nq = idx.shape[0]
    nblk = nq // 128
    out = np.zeros((128, nblk * 64), np.int16)
    for b in range(nblk):
        blk = idx[128 * b:128 * (b + 1)]  # [128, 16]
        tA = blk[0:64].T.astype(np.int16)    # [16, 64]
        tB = blk[64:128].T.astype(np.int16)  # [16, 64]
        for g in range(4):
            out[16 * g:16 * g + 16, 64 * b:64 * b + 64] = tA
            out[64 + 16 * g:64 + 16 * g + 16, 64 * b:64 * b + 64] = tB
    return out


_CACHE = {}
LAST_RESULTS = []


def _programs():
    if "a" not in _CACHE:
        _CACHE["a"] = build_launch_a()
        _CACHE["b"] = build_launch_b()
    return _CACHE["a"], _CACHE["b"]


def kernel(pc1, pc2, feat1, feat2,
           t11_w, t11_b, t22_w, t22_b,
           pos1_w, pos1_b, mlp1_w1, mlp1_b1, mlp1_w2, mlp1_b2,
           t1_w, t1_b, t2_w, t2_b,
           pos2_w, pos2_b, mlp2_w1, mlp2_b1):
    from concourse.bass_utils import run_bass_kernel_spmd

    nca, ncb = _programs()
    B, N = pc1.shape[0], pc1.shape[2]
    nq = N // 2

    common = {
        "t11_wT": np.ascontiguousarray(t11_w.T),
        "t22_wT": np.ascontiguousarray(t22_w.T),
        "pos_wT": np.ascontiguousarray(pos1_w.T),
        "npos_h": np.ascontiguousarray(-0.5 * pos1_w.T),
        "bias_comb": _tile_bias(t11_b + t22_b + pos1_b),
        "w1s9": _bd(0.9 * mlp1_w1.T), "w1s1": _bd(0.1 * mlp1_w1.T),
        "w2s9": _bd(0.9 * mlp1_w2.T), "w2s1": _bd(0.1 * mlp1_w2.T),
        "b1_t": _tile_bias(mlp1_b1), "b2_t": _tile_bias(mlp1_b2),
        "i128": np.eye(128, dtype=np.float32),
        "rep16": np.tile(np.eye(16, dtype=np.float32), (1, 4)),
        "offs": (np.tile((np.arange(64) // 8 * CHUNK + IDX_OFF)
                         .astype(np.float32), (128, 1))),
    }

    in_maps = []
    for b in range(B):
        for d in range(2):
            qpc = pc1[b] if d == 0 else pc2[b]
            cpc = pc2[b] if d == 0 else pc1[b]
            qf = feat1[b] if d == 0 else feat2[b]
            cf = feat2[b] if d == 0 else feat1[b]
            pw, pb = (t1_w, t1_b) if d == 0 else (t2_w, t2_b)
            qa, ca = _aug_q(qpc), _aug_c(cpc)
            for h in range(2):
                sl = slice(nq * h, nq * h + nq)
                m = dict(common)
                m.update({
                    "q_aug": np.ascontiguousarray(qa[:, sl]),
                    "c_aug": ca,
                    "q_feat": np.ascontiguousarray(qf[:, sl]),
                    "c_feat": cf,
                    "projT_bd": _bd(pw.T), "projb_t": _tile_bias(pb),
                })
                in_maps.append(m)

    _ra = run_bass_kernel_spmd(nca, in_maps, list(range(8)))
    LAST_RESULTS.append(_ra)
    res_a = _ra.results

    fn1 = np.zeros((B, C, N), np.float32)
    fn2 = np.zeros((B, C, N), np.float32)
    idx1 = np.zeros((B, N, K), np.int32)
    ci = 0
    for b in range(B):
        for d in range(2):
            for h in range(2):
                sl = slice(nq * h, nq * h + nq)
                fn = res_a[ci]["fn_out"]
                if d == 0:
                    fn1[b, :, sl] = fn
                    idx1[b, sl, :] = res_a[ci]["idx_out"].astype(np.int32)
                else:
                    fn2[b, :, sl] = fn
                ci += 1

    # ---- launch B: cross-layer 3 ----
    common_b = {
        "pos_wT": np.ascontiguousarray(pos2_w.T),
        "npos_wT": np.ascontiguousarray(-pos2_w.T),
        "posb_t": _tile_bias(pos2_b),
        "w1s9": _bd(0.9 * mlp2_w1.T), "w1s1": _bd(0.1 * mlp2_w1.T),
        "b1_t": _tile_bias(mlp2_b1),
        "i128": np.eye(128, dtype=np.float32),
        "i64": np.eye(C, dtype=np.float32),
    }
    nqb = N // 4
    in_maps_b = []
    for b in range(B):
        for s in range(4):
            sl = slice(nqb * s, nqb * s + nqb)
            m = dict(common_b)
            m.update({
                "tab_feat": fn2[b],
                "c_pc": pc2[b],
                "q_fn": np.ascontiguousarray(fn1[b][:, sl]),
                "q_pc": np.ascontiguousarray(pc1[b][:, sl]),
                "idxw": _wrap_idx(idx1[b, sl, :]),
            })
            in_maps_b.append(m)

    _rb = run_bass_kernel_spmd(ncb, in_maps_b, list(range(8)))
    LAST_RESULTS.append(_rb)
    res_b = _rb.results
    ff = np.zeros((B, C, N), np.float32)
    ci = 0
    for b in range(B):
        for s in range(4):
            ff[b, :, nqb * s:nqb * s + nqb] = res_b[ci]["ff_out"]
            ci += 1

    return fn1, fn2, ff


# revision 17
# speedup vs baseline: 1.1408x; 1.1408x over previous
"""Trainium2 Bass kernel for nn_CrossLayerLight (gnn_message_passing).

Strategy (8 NeuronCores):
  Launch A: 8 cores = 2 batches x 2 directions x 2 query-halves.
    Each core: 4096 queries vs 8192 candidates.
      - PE: -d^2 via augmented K=5 matmul (2x1.x2 - |x1|^2 - |x2|^2)
      - DVE: exact top-16 via per-1024-chunk top-8 (max8+max_index) +
        per-row threshold + masked-index extraction (max8 rounds)
      - GPSIMD ap_gather: channel-major neighbor feature gather
      - PE MLP with block-diagonal weights (two query-halves packed into
        128 partitions); leaky = 0.9*relu + 0.1*identity folded into
        doubled matmuls; max-pool over K directly from PSUM.
    Produces fn1, fn2 (projected) and the (x1->x2) knn indices.
  Launch B: 8 cores = 2 batches x 4 query-shards of cross-layer 3,
    reusing launch-A indices and fn1/fn2.
"""

import sys

sys.path.insert(0, "/opt/trn_rl_repo")

import numpy as np

import concourse.bacc as bacc
import concourse.bass as bass
import concourse.mybir as mybir
from concourse.tile import TileContext

F32 = mybir.dt.float32
F32R = mybir.dt.float32r
I16 = mybir.dt.int16
U32 = mybir.dt.uint32


def _r(ap):
    """fp32r disabled (runtime fault on HW) - plain fp32."""
    return ap

LEAKY = 0.1
K = 16
C = 64
QB = 128          # queries per block (partition dim)
CHUNK = 1024      # candidate chunk for L1 top-8
IDX_OFF = 16384.0  # offset so masked-out candidates (0) never win


def _bd(w):
    """[64,64] -> [128,128] block-diagonal (two copies)."""
    out = np.zeros((128, 128), np.float32)
    out[:64, :64] = w
    out[64:, 64:] = w
    return out


def _tile_bias(b):
    return np.tile(np.asarray(b, np.float32).reshape(-1), 2).reshape(128, 1)


def build_launch_a(nq=4096, ncand=8192):
    """Bass program: distance + topk + cross MLP (2 layers) + projection."""
    nblk = nq // QB
    nchunk = ncand // CHUNK
    ncand8 = 8 * nchunk
    sw = min(512, nq)
    pgrp = min(8, nblk)

    nc = bacc.Bacc("TRN2", target_bir_lowering=False, num_devices=8)

    # inputs
    q_aug = nc.dram_tensor("q_aug", [5, nq], F32, kind="ExternalInput").ap()
    c_aug = nc.dram_tensor("c_aug", [5, ncand], F32, kind="ExternalInput").ap()
    q_feat = nc.dram_tensor("q_feat", [C, nq], F32, kind="ExternalInput").ap()
    c_feat = nc.dram_tensor("c_feat", [C, ncand], F32, kind="ExternalInput").ap()
    t11_wT = nc.dram_tensor("t11_wT", [C, C], F32, kind="ExternalInput").ap()
    t22_wT = nc.dram_tensor("t22_wT", [C, C], F32, kind="ExternalInput").ap()
    pos_wT = nc.dram_tensor("pos_wT", [3, C], F32, kind="ExternalInput").ap()
    npos_h = nc.dram_tensor("npos_h", [3, C], F32, kind="ExternalInput").ap()
    bias_comb = nc.dram_tensor("bias_comb", [128, 1], F32, kind="ExternalInput").ap()
    w1s9 = nc.dram_tensor("w1s9", [128, 128], F32, kind="ExternalInput").ap()
    w1s1 = nc.dram_tensor("w1s1", [128, 128], F32, kind="ExternalInput").ap()
    w2s9 = nc.dram_tensor("w2s9", [128, 128], F32, kind="ExternalInput").ap()
    w2s1 = nc.dram_tensor("w2s1", [128, 128], F32, kind="ExternalInput").ap()
    b1_t = nc.dram_tensor("b1_t", [128, 1], F32, kind="ExternalInput").ap()
    b2_t = nc.dram_tensor("b2_t", [128, 1], F32, kind="ExternalInput").ap()
    projT_bd = nc.dram_tensor("projT_bd", [128, 128], F32, kind="ExternalInput").ap()
    projb_t = nc.dram_tensor("projb_t", [128, 1], F32, kind="ExternalInput").ap()
    i128 = nc.dram_tensor("i128", [128, 128], F32, kind="ExternalInput").ap()
    rep16 = nc.dram_tensor("rep16", [16, C], F32, kind="ExternalInput").ap()
    offs = nc.dram_tensor("offs", [128, ncand8], F32, kind="ExternalInput").ap()

    fn_out = nc.dram_tensor("fn_out", [C, nq], F32, kind="ExternalOutput").ap()
    idx_out = nc.dram_tensor("idx_out", [nq, K], F32, kind="ExternalOutput").ap()

    with TileContext(nc) as tc:
        with (
            tc.tile_pool(name="big", bufs=1) as big,
            tc.tile_pool(name="wpool", bufs=1) as wp,
            tc.tile_pool(name="work", bufs=2) as work,
        ):
            # ---- persistent SBUF ----
            q_aug_sb = big.tile([5, nq], F32, tag="qaug")
            c_aug_sb = big.tile([5, ncand], F32, tag="caug")
            s2_sb = big.tile([128, ncand], F32, tag="s2")
            t1n_sb = big.tile([128, nq], F32, tag="t1n")
            idxw_all = big.tile([128, nblk * C], I16, tag="idxw")

            nc.sync.dma_start(out=q_aug_sb[:], in_=q_aug)
            nc.sync.dma_start(out=c_aug_sb[:], in_=c_aug)

            i128_sb = wp.tile([128, 128], F32, tag="i128")
            rep16_sb = wp.tile([16, C], F32, tag="rep16")
            offs_sb = wp.tile([128, ncand8], F32, tag="offs")
            w1s9_sb = wp.tile([128, 128], F32, tag="w1s9")
            w1s1_sb = wp.tile([128, 128], F32, tag="w1s1")
            w2s9_sb = wp.tile([128, 128], F32, tag="w2s9")
            w2s1_sb = wp.tile([128, 128], F32, tag="w2s1")
            projT_sb = wp.tile([128, 128], F32, tag="projT")
            b1_sb = wp.tile([128, 1], F32, tag="b1")
            b2_sb = wp.tile([128, 1], F32, tag="b2")
            projb_sb = wp.tile([128, 1], F32, tag="projb")
            bcomb_sb = wp.tile([128, 1], F32, tag="bcomb")
            t11_sb = wp.tile([C, C], F32, tag="t11")
            t22_sb = wp.tile([C, C], F32, tag="t22")
            posw_sb = wp.tile([3, C], F32, tag="posw")
            nposh_sb = wp.tile([3, C], F32, tag="nposh")
            for dst, src in [
                (i128_sb, i128), (rep16_sb, rep16), (offs_sb, offs),
                (w1s9_sb, w1s9), (w1s1_sb, w1s1), (w2s9_sb, w2s9),
                (w2s1_sb, w2s1), (projT_sb, projT_bd), (b1_sb, b1_t),
                (b2_sb, b2_t), (projb_sb, projb_t), (bcomb_sb, bias_comb),
                (t11_sb, t11_wT), (t22_sb, t22_wT), (posw_sb, pos_wT),
                (nposh_sb, npos_h),
            ]:
                nc.sync.dma_start(out=dst[:], in_=src)

            # ---- setup: s2 table (cand feat + cand pos proj), both halves ----
            with tc.tile_pool(name="setup", bufs=1) as setup, \
                 tc.tile_pool(name="setps", bufs=2, space="PSUM") as setps:
                c_feat_sb = setup.tile([C, ncand], F32, tag="cfeat")
                q_feat_sb = setup.tile([C, nq], F32, tag="qfeat")
                nc.sync.dma_start(out=c_feat_sb[:], in_=c_feat)
                nc.sync.dma_start(out=q_feat_sb[:], in_=q_feat)
                for j in range(ncand // 512):
                    sl = slice(512 * j, 512 * j + 512)
                    ps = setps.tile([C, 512], F32, tag="sps")
                    nc.tensor.matmul(out=ps[:], lhsT=_r(t22_sb[:]),
                                     rhs=_r(c_feat_sb[:, sl]), start=True, stop=False)
                    nc.tensor.matmul(out=ps[:], lhsT=_r(posw_sb[:]),
                                     rhs=_r(c_aug_sb[0:3, sl]), start=False, stop=True)
                    nc.vector.tensor_copy(out=s2_sb[0:64, sl], in_=ps[:])
                # t1n = t11(q_feat) - 0.5*pos( 2x1 ) + bias_comb
                for j in range(nq // sw):
                    sl = slice(sw * j, sw * j + sw)
                    ps = setps.tile([C, sw], F32, tag="sps")
                    nc.tensor.matmul(out=ps[:], lhsT=_r(t11_sb[:]),
                                     rhs=_r(q_feat_sb[:, sl]), start=True, stop=False)
                    nc.tensor.matmul(out=ps[:], lhsT=_r(nposh_sb[:]),
                                     rhs=_r(q_aug_sb[0:3, sl]), start=False, stop=True)
                    nc.vector.tensor_scalar(out=t1n_sb[0:64, sl], in0=ps[:],
                                            scalar1=bcomb_sb[0:64, :], scalar2=None,
                                            op0=mybir.AluOpType.add)
            # replicate to partitions 64:128 (t1n shifted by 64 queries)
            nc.sync.dma_start(out=s2_sb[64:128, :], in_=s2_sb[0:64, :])
            nc.sync.dma_start(out=t1n_sb[64:128, 0:nq - 64], in_=t1n_sb[0:64, 64:nq])

            mainps = tc.tile_pool(name="distp", bufs=2, space="PSUM")
            distp = mainps.__enter__()
            mlpp_cm = tc.tile_pool(name="mlpp", bufs=2, space="PSUM")
            mlpp = mlpp_cm.__enter__()
            smallp_cm = tc.tile_pool(name="smallp", bufs=2, space="PSUM")
            smallp = smallp_cm.__enter__()
            pooled_col = work.tile([128, C * pgrp], F32, tag="pooled")
            gat_tiles = {}
            pooled_box = [pooled_col]

            def phase1(bi):
                q0 = QB * bi
                lhs_q = q_aug_sb[:, q0:q0 + QB]
                # --- distances + L1 top8 per chunk ---
                vals = work.tile([128, ncand8], F32, tag="vals")
                pidx = work.tile([128, ncand8], U32, tag="pidx")
                for cki in range(nchunk):
                    dps = distp.tile([128, CHUNK], F32, tag="dps")
                    for h in range(CHUNK // 512):
                        c0 = CHUNK * cki + 512 * h
                        nc.tensor.matmul(out=dps[:, 512 * h:512 * h + 512],
                                         lhsT=lhs_q,
                                         rhs=c_aug_sb[:, c0:c0 + 512],
                                         start=True, stop=True)
                    nc.vector.max(out=vals[:, 8 * cki:8 * cki + 8], in_=dps[:])
                    nc.vector.max_index(out=pidx[:, 8 * cki:8 * cki + 8],
                                        in_max=vals[:, 8 * cki:8 * cki + 8],
                                        in_values=dps[:])
                # --- L2: threshold = 16th largest of the L1 candidates ---
                gidx = work.tile([128, ncand8], F32, tag="gidx")
                nc.vector.tensor_copy(out=gidx[:], in_=pidx[:])
                nc.vector.tensor_tensor(out=gidx[:], in0=gidx[:], in1=offs_sb[:],
                                        op=mybir.AluOpType.add)
                r8 = work.tile([128, 16], F32, tag="r8")
                vrep = work.tile([128, ncand8], F32, tag="vrep")
                nc.vector.max(out=r8[:, 0:8], in_=vals[:])
                nc.vector.match_replace(out=vrep[:], in_to_replace=r8[:, 0:8],
                                        in_values=vals[:], imm_value=-3.0e38)
                nc.vector.max(out=r8[:, 8:16], in_=vrep[:])
                # mask of >= T ; masked global indices (invalid -> 0)
                msk = work.tile([128, ncand8], F32, tag="msk")
                nc.vector.tensor_scalar(out=msk[:], in0=vals[:],
                                        scalar1=r8[:, 15:16], scalar2=None,
                                        op0=mybir.AluOpType.is_ge)
                mgi = work.tile([128, ncand8], F32, tag="mgi")
                nc.vector.tensor_tensor(out=mgi[:], in0=msk[:], in1=gidx[:],
                                        op=mybir.AluOpType.mult)
                e16 = work.tile([128, 16], F32, tag="e16")
                nc.vector.max(out=e16[:, 0:8], in_=mgi[:])
                nc.vector.match_replace(out=mgi[:], in_to_replace=e16[:, 0:8],
                                        in_values=mgi[:], imm_value=0.0)
                nc.vector.max(out=e16[:, 8:16], in_=mgi[:])
                idx16 = work.tile([128, 16], F32, tag="idx16")
                nc.vector.tensor_scalar(out=idx16[:], in0=e16[:],
                                        scalar1=-IDX_OFF, scalar2=None,
                                        op0=mybir.AluOpType.add)
                nc.sync.dma_start(out=idx_out[q0:q0 + QB, :], in_=idx16[:])
                # --- wrap indices into gpsimd layout: [16k, q] replicated 4x ---
                trp = smallp.tile([16, 128], F32, tag="sm")
                nc.tensor.transpose(out=trp[:], in_=idx16[:], identity=i128_sb[:])
                trs = work.tile([16, 128], F32, tag="trs")
                nc.vector.tensor_copy(out=trs[:], in_=trp[:])
                iwp = smallp.tile([128, C], F32, tag="sm")
                nc.tensor.matmul(out=iwp[0:64, :], lhsT=rep16_sb[:],
                                 rhs=trs[:, 0:64], start=True, stop=True)
                nc.tensor.matmul(out=iwp[64:128, :], lhsT=rep16_sb[:],
                                 rhs=trs[:, 64:128], start=True, stop=True,
                                 tile_position=(0, 64))
                nc.vector.tensor_copy(out=idxw_all[:, C * bi:C * bi + C], in_=iwp[:])
                # --- gather neighbor features (channel-major, both halves) ---
                gat = work.tile([128, CHUNK], F32, tag="gat")
                nc.gpsimd.ap_gather(out_ap=gat[:], in_ap=s2_sb[:],
                                    idxs_ap=idxw_all[:, C * bi:C * bi + C],
                                    channels=128, num_elems=ncand, d=1,
                                    num_idxs=CHUNK)
                gat_tiles[bi] = gat

            def phase2(bi):
                q0 = QB * bi
                gat = gat_tiles.pop(bi)
                pooled_col = pooled_box[0]
                # --- MLP ---
                t1n_rep = (t1n_sb[:, q0:q0 + 64]
                           .unsqueeze(2).broadcast_to([128, 64, 16]))
                for h in range(2):
                    fs = slice(512 * h, 512 * h + 512)
                    u1 = mlpp.tile([128, 512], F32, tag="u")
                    nc.tensor.matmul(out=u1[:], lhsT=_r(i128_sb[:]),
                                     rhs=_r(gat[:, fs]), start=True, stop=False)
                    rr = t1n_rep[:, 32 * h:32 * h + 32, :]
                    nc.tensor.matmul(out=u1[:], lhsT=_r(i128_sb[:]), rhs=_r(rr),
                                     start=False, stop=True)
                    r1 = work.tile([128, 512], F32, tag="r1")
                    c1 = work.tile([128, 512], F32, tag="c1")
                    nc.scalar.activation(r1[:], u1[:],
                                         mybir.ActivationFunctionType.Relu)
                    nc.scalar.activation(c1[:], u1[:],
                                         mybir.ActivationFunctionType.Copy)
                    u2 = mlpp.tile([128, 512], F32, tag="u")
                    nc.tensor.matmul(out=u2[:], lhsT=_r(w1s9_sb[:]), rhs=_r(r1[:]),
                                     start=True, stop=False)
                    nc.tensor.matmul(out=u2[:], lhsT=_r(w1s1_sb[:]), rhs=_r(c1[:]),
                                     start=False, stop=True)
                    r2 = work.tile([128, 512], F32, tag="r2")
                    c2 = work.tile([128, 512], F32, tag="c2")
                    nc.scalar.activation(r2[:], u2[:],
                                         mybir.ActivationFunctionType.Relu,
                                         bias=b1_sb[:, :])
                    nc.scalar.activation(c2[:], u2[:],
                                         mybir.ActivationFunctionType.Identity,
                                         bias=b1_sb[:, :])
                    u3 = mlpp.tile([128, 512], F32, tag="u")
                    nc.tensor.matmul(out=u3[:], lhsT=_r(w2s9_sb[:]), rhs=_r(r2[:]),
                                     start=True, stop=False)
                    nc.tensor.matmul(out=u3[:], lhsT=_r(w2s1_sb[:]), rhs=_r(c2[:]),
                                     start=False, stop=True)
                    # max-pool over K directly from PSUM, then leaky(x + b2)
                    pp = work.tile([128, 32], F32, tag="pp")
                    nc.vector.tensor_reduce(out=pp[:],
                                            in_=u3[:].rearrange("p (q k) -> p q k",
                                                                k=16),
                                            axis=mybir.AxisListType.X,
                                            op=mybir.AluOpType.max)
                    pb = work.tile([128, 32], F32, tag="pb")
                    nc.vector.tensor_scalar(out=pb[:], in0=pp[:],
                                            scalar1=b2_sb[:, :], scalar2=None,
                                            op0=mybir.AluOpType.add)
                    po = pooled_col[:, C * (bi % pgrp) + 32 * h:
                                    C * (bi % pgrp) + 32 * h + 32]
                    nc.vector.tensor_scalar(out=po, in0=pb[:],
                                            scalar1=LEAKY, scalar2=None,
                                            op0=mybir.AluOpType.mult)
                    nc.vector.tensor_tensor(out=po, in0=po, in1=pb[:],
                                            op=mybir.AluOpType.max)
                # --- projection every pgrp blocks ---
                if bi % pgrp == pgrp - 1:
                    g8 = bi // pgrp
                    prj = smallp.tile([128, C * pgrp], F32, tag="sm")
                    nc.tensor.matmul(out=prj[:], lhsT=_r(projT_sb[:]),
                                     rhs=_r(pooled_col[:]), start=True, stop=True)
                    fno = work.tile([128, C * pgrp], F32, tag="fno")
                    nc.vector.tensor_scalar(out=fno[:], in0=prj[:],
                                            scalar1=projb_sb[:, :], scalar2=None,
                                            op0=mybir.AluOpType.add)
                    # halves interleave: block b covers queries [128b,128b+128):
                    # partitions 0:64 -> q 128b..+64 ; 64:128 -> q 128b+64..+128
                    dstA = (fn_out[:, QB * pgrp * g8:QB * pgrp * (g8 + 1)]
                            .rearrange("c (b q) -> c b q", q=QB)[:, :, 0:64])
                    dstB = (fn_out[:, QB * pgrp * g8:QB * pgrp * (g8 + 1)]
                            .rearrange("c (b q) -> c b q", q=QB)[:, :, 64:128])
                    srcA = fno[0:64, :].rearrange("c (b q) -> c b q", q=64)
                    srcB = fno[64:128, :].rearrange("c (b q) -> c b q", q=64)
                    nc.sync.dma_start(out=dstA, in_=srcA)
                    nc.sync.dma_start(out=dstB, in_=srcB)
                    pooled_new = work.tile([128, C * pgrp], F32, tag="pooled")
                    pooled_box[0] = pooled_new

            for bi in range(nblk):
                phase1(bi)
                phase2(bi)
            smallp_cm.__exit__(None, None, None)
            mlpp_cm.__exit__(None, None, None)
            mainps.__exit__(None, None, None)
    nc.compile()
    return nc


def build_launch_b(nq=2048, ncand=8192):
    """Bass program: cross-layer 3 (gather + 1 mlp layer + pool)."""
    nblk = nq // QB
    sw = min(512, nq)
    pgrp = min(8, nblk)
    nc = bacc.Bacc("TRN2", target_bir_lowering=False, num_devices=8)

    tab_feat = nc.dram_tensor("tab_feat", [C, ncand], F32, kind="ExternalInput").ap()
    c_pc = nc.dram_tensor("c_pc", [3, ncand], F32, kind="ExternalInput").ap()
    q_fn = nc.dram_tensor("q_fn", [C, nq], F32, kind="ExternalInput").ap()
    q_pc = nc.dram_tensor("q_pc", [3, nq], F32, kind="ExternalInput").ap()
    idxw = nc.dram_tensor("idxw", [128, nq // QB * C], I16,
                          kind="ExternalInput").ap()
    pos_wT = nc.dram_tensor("pos_wT", [3, C], F32, kind="ExternalInput").ap()
    npos_wT = nc.dram_tensor("npos_wT", [3, C], F32, kind="ExternalInput").ap()
    posb_t = nc.dram_tensor("posb_t", [128, 1], F32, kind="ExternalInput").ap()
    w1s9 = nc.dram_tensor("w1s9", [128, 128], F32, kind="ExternalInput").ap()
    w1s1 = nc.dram_tensor("w1s1", [128, 128], F32, kind="ExternalInput").ap()
    b1_t = nc.dram_tensor("b1_t", [128, 1], F32, kind="ExternalInput").ap()
    i128 = nc.dram_tensor("i128", [128, 128], F32, kind="ExternalInput").ap()
    i64 = nc.dram_tensor("i64", [C, C], F32, kind="ExternalInput").ap()

    ff_out = nc.dram_tensor("ff_out", [C, nq], F32, kind="ExternalOutput").ap()

    with TileContext(nc) as tc:
        with (
            tc.tile_pool(name="big", bufs=1) as big,
            tc.tile_pool(name="wpool", bufs=1) as wp,
            tc.tile_pool(name="work", bufs=2) as work,
            tc.tile_pool(name="mlpp", bufs=2, space="PSUM") as mlpp,
        ):
            tab_sb = big.tile([128, ncand], F32, tag="tab")
            t1n_sb = big.tile([128, nq], F32, tag="t1n")
            idxw_sb = big.tile([128, nblk * C], I16, tag="idxw")
            ffc = big.tile([128, C * pgrp], F32, tag="ffc")
            nc.sync.dma_start(out=idxw_sb[:], in_=idxw)

            i128_sb = wp.tile([128, 128], F32, tag="i128")
            i64_sb = wp.tile([C, C], F32, tag="i64")
            w1s9_sb = wp.tile([128, 128], F32, tag="w1s9")
            w1s1_sb = wp.tile([128, 128], F32, tag="w1s1")
            b1_sb = wp.tile([128, 1], F32, tag="b1")
            posb_sb = wp.tile([128, 1], F32, tag="posb")
            posw_sb = wp.tile([3, C], F32, tag="posw")
            nposw_sb = wp.tile([3, C], F32, tag="nposw")
            for dst, src in [(i128_sb, i128), (i64_sb, i64), (w1s9_sb, w1s9),
                             (w1s1_sb, w1s1), (b1_sb, b1_t), (posb_sb, posb_t),
                             (posw_sb, pos_wT), (nposw_sb, npos_wT)]:
                nc.sync.dma_start(out=dst[:], in_=src)

            with tc.tile_pool(name="setup", bufs=1) as setup, \
                 tc.tile_pool(name="setps", bufs=2, space="PSUM") as setps:
                tf_sb = setup.tile([C, ncand], F32, tag="tf")
                cpc_sb = setup.tile([3, ncand], F32, tag="cpc")
                qfn_sb = setup.tile([C, nq], F32, tag="qfn")
                qpc_sb = setup.tile([3, nq], F32, tag="qpc")
                nc.sync.dma_start(out=tf_sb[:], in_=tab_feat)
                nc.sync.dma_start(out=cpc_sb[:], in_=c_pc)
                nc.sync.dma_start(out=qfn_sb[:], in_=q_fn)
                nc.sync.dma_start(out=qpc_sb[:], in_=q_pc)
                for j in range(ncand // 512):
                    sl = slice(512 * j, 512 * j + 512)
                    ps = setps.tile([C, 512], F32, tag="sps")
                    nc.tensor.matmul(out=ps[:], lhsT=_r(i64_sb[:]),
                                     rhs=_r(tf_sb[:, sl]), start=True, stop=False)
                    nc.tensor.matmul(out=ps[:], lhsT=_r(posw_sb[:]),
                                     rhs=_r(cpc_sb[:, sl]), start=False, stop=True)
                    nc.vector.tensor_copy(out=tab_sb[0:64, sl], in_=ps[:])
                for j in range(nq // sw):
                    sl = slice(sw * j, sw * j + sw)
                    ps = setps.tile([C, sw], F32, tag="sps")
                    nc.tensor.matmul(out=ps[:], lhsT=_r(i64_sb[:]),
                                     rhs=_r(qfn_sb[:, sl]), start=True, stop=False)
                    nc.tensor.matmul(out=ps[:], lhsT=_r(nposw_sb[:]),
                                     rhs=_r(qpc_sb[:, sl]), start=False, stop=True)
                    nc.vector.tensor_scalar(out=t1n_sb[0:64, sl], in0=ps[:],
                                            scalar1=posb_sb[0:64, :], scalar2=None,
                                            op0=mybir.AluOpType.add)
            nc.sync.dma_start(out=tab_sb[64:128, :], in_=tab_sb[0:64, :])
            nc.sync.dma_start(out=t1n_sb[64:128, 0:nq - 64], in_=t1n_sb[0:64, 64:nq])

            for bi in range(nblk):
                q0 = QB * bi
                gat = work.tile([128, CHUNK], F32, tag="gat")
                nc.gpsimd.ap_gather(out_ap=gat[:], in_ap=tab_sb[:],
                                    idxs_ap=idxw_sb[:, C * bi:C * bi + C],
                                    channels=128, num_elems=ncand, d=1,
                                    num_idxs=CHUNK)
                t1n_rep = (t1n_sb[:, q0:q0 + 64]
                           .unsqueeze(2).broadcast_to([128, 64, 16]))
                for h in range(2):
                    fs = slice(512 * h, 512 * h + 512)
                    u1 = mlpp.tile([128, 512], F32, tag="u")
                    nc.tensor.matmul(out=u1[:], lhsT=_r(i128_sb[:]),
                                     rhs=_r(gat[:, fs]), start=True, stop=False)
                    rr = t1n_rep[:, 32 * h:32 * h + 32, :]
                    nc.tensor.matmul(out=u1[:], lhsT=_r(i128_sb[:]), rhs=_r(rr),
                                     start=False, stop=True)
                    r1 = work.tile([128, 512], F32, tag="r1")
                    c1 = work.tile([128, 512], F32, tag="c1")
                    nc.scalar.activation(r1[:], u1[:],
                                         mybir.ActivationFunctionType.Relu)
                    nc.scalar.activation(c1[:], u1[:],
                                         mybir.ActivationFunctionType.Copy)
                    u2 = mlpp.tile([128, 512], F32, tag="u")
                    nc.tensor.matmul(out=u2[:], lhsT=_r(w1s9_sb[:]), rhs=_r(r1[:]),
                                     start=True, stop=False)
                    nc.tensor.matmul(out=u2[:], lhsT=_r(w1s1_sb[:]), rhs=_r(c1[:]),
                                     start=False, stop=True)
                    pp = work.tile([128, 32], F32, tag="pp")
                    nc.vector.tensor_reduce(out=pp[:],
                                            in_=u2[:].rearrange("p (q k) -> p q k",
                                                                k=16),
                                            axis=mybir.AxisListType.X,
                                            op=mybir.AluOpType.max)
                    pb = work.tile([128, 32], F32, tag="pb")
                    nc.vector.tensor_scalar(out=pb[:], in0=pp[:],
                                            scalar1=b1_sb[:, :], scalar2=None,
                                            op0=mybir.AluOpType.add)
                    po = ffc[:, C * (bi % pgrp) + 32 * h:
                             C * (bi % pgrp) + 32 * h + 32]
                    nc.vector.tensor_scalar(out=po, in0=pb[:], scalar1=LEAKY,
                                            scalar2=None, op0=mybir.AluOpType.mult)
                    nc.vector.tensor_tensor(out=po, in0=po, in1=pb[:],
                                            op=mybir.AluOpType.max)
                if bi % pgrp == pgrp - 1:
                    g8 = bi // pgrp
                    dstA = (ff_out[:, QB * pgrp * g8:QB * pgrp * (g8 + 1)]
                            .rearrange("c (b q) -> c b q", q=QB)[:, :, 0:64])
                    dstB = (ff_out[:, QB * pgrp * g8:QB * pgrp * (g8 + 1)]
                            .rearrange("c (b q) -> c b q", q=QB)[:, :, 64:128])
                    srcA = ffc[0:64, :].rearrange("c (b q) -> c b q", q=64)
                    srcB = ffc[64:128, :].rearrange("c (b q) -> c b q", q=64)
                    nc.sync.dma_start(out=dstA, in_=srcA)
                    nc.sync.dma_start(out=dstB, in_=srcB)
                    ffc = big.tile([128, C * pgrp], F32, tag="ffc")
    nc.compile()
    return nc


# ------------------------- host orchestration -------------------------

def _aug_q(pc):
    # [3, n] -> [5, n]: rows 2x,2y,2z, -|x|^2, 1
    n2 = (pc.astype(np.float32) ** 2).sum(axis=0)
    return np.concatenate([2.0 * pc, -n2[None, :],
                           np.ones_like(n2)[None, :]], axis=0).astype(np.float32)


def _aug_c(pc):
    # [3, n] -> [5, n]: rows x,y,z, 1, -|x|^2
    n2 = (pc.astype(np.float32) ** 2).sum(axis=0)
    return np.concatenate([pc, np.ones_like(n2)[None, :],
                           -n2[None, :]], axis=0).astype(np.float32)


def _wrap_idx(idx):
    """[nq, 16] int -> gpsimd wrapped layout [128, nq*16//2048*64] int16.

    Block b (128 queries): col block [64b, 64b+64); partition 16g+p,
    col s of block: g in 0..3 -> idx[128b + s, p] ; g in 4..7 ->
    idx[128b + 64 + s, p].
    """
    nq = idx.shape[0]
    nblk = nq // 128
    out = np.zeros((128, nblk * 64), np.int16)
    for b in range(nblk):
        blk = idx[128 * b:128 * (b + 1)]  # [128, 16]
        tA = blk[0:64].T.astype(np.int16)    # [16, 64]
        tB = blk[64:128].T.astype(np.int16)  # [16, 64]
        for g in range(4):
            out[16 * g:16 * g + 16, 64 * b:64 * b + 64] = tA
            out[64 + 16 * g:64 + 16 * g + 16, 64 * b:64 * b + 64] = tB
    return out


_CACHE = {}
LAST_RESULTS = []


def _programs():
    if "a" not in _CACHE:
        _CACHE["a"] = build_launch_a()
        _CACHE["b"] = build_launch_b()
    return _CACHE["a"], _CACHE["b"]


def kernel(pc1, pc2, feat1, feat2,
           t11_w, t11_b, t22_w, t22_b,
           pos1_w, pos1_b, mlp1_w1, mlp1_b1, mlp1_w2, mlp1_b2,
           t1_w, t1_b, t2_w, t2_b,
           pos2_w, pos2_b, mlp2_w1, mlp2_b1):
    from concourse.bass_utils import run_bass_kernel_spmd

    nca, ncb = _programs()
    B, N = pc1.shape[0], pc1.shape[2]
    nq = N // 2

    common = {
        "t11_wT": np.ascontiguousarray(t11_w.T),
        "t22_wT": np.ascontiguousarray(t22_w.T),
        "pos_wT": np.ascontiguousarray(pos1_w.T),
        "npos_h": np.ascontiguousarray(-0.5 * pos1_w.T),
        "bias_comb": _tile_bias(t11_b + t22_b + pos1_b),
        "w1s9": _bd(0.9 * mlp1_w1.T), "w1s1": _bd(0.1 * mlp1_w1.T),
        "w2s9": _bd(0.9 * mlp1_w2.T), "w2s1": _bd(0.1 * mlp1_w2.T),
        "b1_t": _tile_bias(mlp1_b1), "b2_t": _tile_bias(mlp1_b2),
        "i128": np.eye(128, dtype=np.float32),
        "rep16": np.tile(np.eye(16, dtype=np.float32), (1, 4)),
        "offs": (np.tile((np.arange(64) // 8 * CHUNK + IDX_OFF)
                         .astype(np.float32), (128, 1))),
    }

    in_maps = []
    for b in range(B):
        for d in range(2):
            qpc = pc1[b] if d == 0 else pc2[b]
            cpc = pc2[b] if d == 0 else pc1[b]
            qf = feat1[b] if d == 0 else feat2[b]
            cf = feat2[b] if d == 0 else feat1[b]
            pw, pb = (t1_w, t1_b) if d == 0 else (t2_w, t2_b)
            qa, ca = _aug_q(qpc), _aug_c(cpc)
            for h in range(2):
                sl = slice(nq * h, nq * h + nq)
                m = dict(common)
                m.update({
                    "q_aug": np.ascontiguousarray(qa[:, sl]),
                    "c_aug": ca,
                    "q_feat": np.ascontiguousarray(qf[:, sl]),
                    "c_feat": cf,
                    "projT_bd": _bd(pw.T), "projb_t": _tile_bias(pb),
                })
                in_maps.append(m)

    _ra = run_bass_kernel_spmd(nca, in_maps, list(range(8)))
    LAST_RESULTS.append(_ra)
    res_a = _ra.results

    fn1 = np.zeros((B, C, N), np.float32)
    fn2 = np.zeros((B, C, N), np.float32)
    idx1 = np.zeros((B, N, K), np.int32)
    ci = 0
    for b in range(B):
        for d in range(2):
            for h in range(2):
                sl = slice(nq * h, nq * h + nq)
                fn = res_a[ci]["fn_out"]
                if d == 0:
                    fn1[b, :, sl] = fn
                    idx1[b, sl, :] = res_a[ci]["idx_out"].astype(np.int32)
                else:
                    fn2[b, :, sl] = fn
                ci += 1

    # ---- launch B: cross-layer 3 ----
    common_b = {
        "pos_wT": np.ascontiguousarray(pos2_w.T),
        "npos_wT": np.ascontiguousarray(-pos2_w.T),
        "posb_t": _tile_bias(pos2_b),
        "w1s9": _bd(0.9 * mlp2_w1.T), "w1s1": _bd(0.1 * mlp2_w1.T),
        "b1_t": _tile_bias(mlp2_b1),
        "i128": np.eye(128, dtype=np.float32),
        "i64": np.eye(C, dtype=np.float32),
    }
    nqb = N // 4
    in_maps_b = []
    for b in range(B):
        for s in range(4):
            sl = slice(nqb * s, nqb * s + nqb)
            m = dict(common_b)
            m.update({
                "tab_feat": fn2[b],
                "c_pc": pc2[b],
                "q_fn": np.ascontiguousarray(fn1[b][:, sl]),
                "q_pc": np.ascontiguousarray(pc1[b][:, sl]),
                "idxw": _wrap_idx(idx1[b, sl, :]),
            })
            in_maps_b.append(m)

    _rb = run_bass_kernel_spmd(ncb, in_maps_b, list(range(8)))
    LAST_RESULTS.append(_rb)
    res_b = _rb.results
    ff = np.zeros((B, C, N), np.float32)
    ci = 0
    for b in range(B):
        for s in range(4):
            ff[b, :, nqb * s:nqb * s + nqb] = res_b[ci]["ff_out"]
            ci += 1

    return fn1, fn2, ff
